# revision 67
# baseline (speedup 1.0000x reference)
"""AttentionBlock (GroupNorm -> QKV -> full attention -> out-proj + residual)
for B=4, C=128, N=4096 on 8 Trainium2 NeuronCores.

Sharding: 8 cores = 4 batches x 2 query-slabs of N/2. Every core runs the
same program; the host rolls each core's x so its query slab is always
columns [0, N/2).

Key moves:
- q/k are never materialized: scores = h^T (w_q^T w_k) h + h^T (w_k^T b_q)
  with the weight product composed on the host, and the k-bias dropped
  (softmax is invariant to per-query constants). One slab projection
  qt = M^T h + bqt feeds all QK matmuls with h itself as the stationary side.
- Matmuls run in float32r (fp32 data, PE rounds to ~tf32 at full speed);
  exp'd probabilities are stored fp8e4m3 and the PV matmul contracts two
  128-key tiles per instruction with fp8 DoubleRow (2x PE throughput).
- Scores are computed transposed [j, i] so exp feeds PV with no transposes;
  softmax row sums come from all-ones matmuls over DVE-pairsummed P tiles
  accumulated in PSUM next to PV, normalized at the end of each pass by
  reciprocal_approx_fast + one multiply.
- PE executes in program order, so PV/rowsum work for a tile pair is emitted
  one tile late, keeping PE busy while ACT runs exp (software pipelining).
- The v projection absorbs the GroupNorm affine (vT = xB^T (a o w_v), with the
  shift folded through attention into the output bias), so the vT pipeline
  runs off raw bf16 x during the stats phase instead of waiting for h.
End-to-end relative error vs the fp32 reference is ~2e-4 (fp8-dominated);
cost-model (TimelineSim) per-core time ~94us.
"""

import math
import sys

if "/opt/trn_rl_repo" not in sys.path:
    sys.path.insert(0, "/opt/trn_rl_repo")

import numpy as np

C = 128
G = 8
GS = C // G  # channels per group
EPS = 1e-5
N_CORES = 8


def build(N=4096, repeat=1):
    """Build the per-core Bass program. Returns the compiled Bacc module."""
    import concourse.bacc as bacc
    import concourse.bass as bass
    import concourse.mybir as mybir
    import concourse.tile as tile

    f32 = mybir.dt.float32
    f32r = mybir.dt.float32r
    AF = mybir.ActivationFunctionType
    OP = mybir.AluOpType

    S = N // 2           # query slab width per core
    ICW = min(1024, S)   # i-chunk width (one PV/rowsum accumulation pass)
    NIC = S // ICW       # number of i-chunk passes
    NJT = N // 128       # number of j (key) tiles
    BNC = min(512, N)    # bn_stats chunk
    NBN = N // BNC
    PCW = min(512, S)    # projection/epilogue chunk width for slab-sized tensors
    NPC = S // PCW
    SCALE = 1.0 / math.sqrt(C)

    nc = bacc.Bacc("TRN2", target_bir_lowering=False, debug=False)

    x_d = nc.dram_tensor("x", [C, N], f32, kind="ExternalInput").ap()
    w_d = nc.dram_tensor("wcat", [C, 4 * C], f32, kind="ExternalInput").ap()
    m_d = nc.dram_tensor("gmask", [C, C], f32, kind="ExternalInput").ap()
    b_d = nc.dram_tensor("bcat", [C, 5], f32, kind="ExternalInput").ap()
    o_d = nc.dram_tensor("out", [C, S], f32, kind="ExternalOutput").ap()

    with tile.TileContext(nc) as tc:
        with tc.tile_pool(name="consts", bufs=1) as cp, \
             tc.tile_pool(name="big", bufs=1) as bp, \
             tc.tile_pool(name="small", bufs=3) as sp_, \
             tc.tile_pool(name="pP", bufs=6) as pP:
            _loop = tc.For_i(0, repeat, 1) if repeat > 1 else None
            if _loop is not None:
                _loop.__enter__()

            # ---- loads + constants ----
            bf16 = mybir.dt.bfloat16
            xS = bp.tile([C, N], f32, tag="x")
            for dc in range(NBN):
                nc.sync.dma_start(xS[:, dc * BNC:(dc + 1) * BNC],
                                  x_d[:, dc * BNC:(dc + 1) * BNC])
            xB = bp.tile([C, N], bf16, tag="xB")
            for dc in range(NBN):
                nc.gpsimd.tensor_copy(out=xB[:, dc * BNC:(dc + 1) * BNC],
                                      in_=xS[:, dc * BNC:(dc + 1) * BNC])
            wS = cp.tile([C, 4 * C], f32, tag="w")
            nc.sync.dma_start(wS[:], w_d[:])
            wR = cp.tile([C, 4 * C], f32r, tag="wr")
            nc.vector.tensor_copy(wR[:], wS[:])
            mS = cp.tile([C, C], f32, tag="gmask")
            nc.sync.dma_start(mS[:], m_d[:])
            bS = cp.tile([C, 5], f32, tag="bcat")
            nc.sync.dma_start(bS[:], b_d[:])
            onesS = cp.tile([C, C], f32, tag="ones")
            nc.vector.memset(onesS[:], 1.0)
            onesR = cp.tile([C, C], f32r, tag="onesr")
            nc.vector.tensor_copy(onesR[:], onesS[:])
            epsT = cp.tile([C, 1], f32, tag="eps")
            nc.vector.memset(epsT[:], EPS)
            f8 = mybir.dt.float8e4
            onesF8 = cp.tile([C, C], f8, tag="onesf8")
            nc.vector.tensor_copy(onesF8[:], onesS[:])

            hR = bp.tile([C, N], f32r, tag="h")
            qtR = bp.tile([C, S], f32r, tag="qt")
            vTR = bp.tile([C, N], f8, tag="vT")
            h2nR = bp.tile([C, S], f32r, tag="h2n")
            outS = bp.tile([C, S], f32, tag="outS")

            with tc.tile_pool(name="ps_pre", bufs=2, space="PSUM") as pre, \
                 tc.tile_pool(name="ps_vt", bufs=2, space="PSUM") as pvt:
                # ---- GroupNorm stats ----
                st6 = sp_.tile([C, NBN, 6], f32, tag="st6")
                for i in range(NBN):
                    nc.vector.bn_stats(out=st6[:, i, :], in_=xS[:, i * BNC:(i + 1) * BNC])
                mv = sp_.tile([C, 2], f32, tag="mv")
                nc.vector.bn_aggr(out=mv[:], in_=st6[:])
                # mv col1 <- mean^2 + var = E[x^2] (in place)
                nc.vector.scalar_tensor_tensor(out=mv[:, 1:2], in0=mv[:, 0:1],
                                               scalar=mv[:, 0:1], in1=mv[:, 1:2],
                                               op0=OP.mult, op1=OP.add)
                # cross-partition group reduce: gstats[c,:] = [gmean, gEx2] of c's group
                gps = pre.tile([C, 2], f32, tag="gstats")
                nc.tensor.matmul(gps[:], mS[:], mv[:], start=True, stop=True)
                gst = sp_.tile([C, 2], f32, tag="gst")
                nc.vector.tensor_copy(gst[:], gps[:])
                # xv = eps + gEx2 - gmean^2  (group variance + eps)
                i32 = mybir.dt.int32
                gv = sp_.tile([C, 1], f32, tag="gv")
                nc.vector.scalar_tensor_tensor(out=gv[:], in0=gst[:, 0:1],
                                               scalar=gst[:, 0:1], in1=gst[:, 1:2],
                                               op0=OP.mult, op1=OP.subtract)
                xv = sp_.tile([C, 1], f32, tag="xv")
                nc.vector.tensor_tensor(out=xv[:], in0=epsT[:], in1=gv[:], op=OP.subtract)
                magicT = cp.tile([C, 1], i32, tag="magic")
                nc.vector.memset(magicT[:], 0x5F3759DF)
                yh = sp_.tile([C, 1], i32, tag="yh")
                nc.vector.tensor_scalar(out=yh[:], in0=xv[:].bitcast(i32), scalar1=1,
                                        scalar2=None, op0=OP.logical_shift_right)
                nc.vector.tensor_tensor(out=yh[:], in0=magicT[:], in1=yh[:], op=OP.subtract)
                inv = sp_.tile([C, 1], f32, tag="inv")
                nc.vector.tensor_copy(inv[:], yh[:].bitcast(f32))
                tN = sp_.tile([C, 1], f32, tag="tN")
                for _ in range(2):
                    nc.vector.tensor_tensor(out=tN[:], in0=inv[:], in1=inv[:], op=OP.mult)
                    nc.vector.tensor_tensor(out=tN[:], in0=tN[:], in1=xv[:], op=OP.mult)
                    nc.vector.tensor_scalar(out=tN[:], in0=tN[:], scalar1=-0.5,
                                            scalar2=1.5, op0=OP.mult, op1=OP.add)
                    nc.vector.tensor_tensor(out=inv[:], in0=inv[:], in1=tN[:], op=OP.mult)
                aT = sp_.tile([C, 1], f32, tag="aT")
                nc.vector.tensor_tensor(out=aT[:], in0=bS[:, 3:4], in1=inv[:], op=OP.mult)
                bT = sp_.tile([C, 1], f32, tag="bT")
                nc.vector.tensor_tensor(out=bT[:], in0=gst[:, 0:1], in1=aT[:], op=OP.mult)
                nc.vector.tensor_tensor(out=bT[:], in0=bS[:, 4:5], in1=bT[:], op=OP.subtract)
                # v absorbs the GN affine: v = (a.w_v)^T x + w_v^T b; the
                # constant term rides through attention as a per-channel
                # offset on h2n and folds into the output bias via W_out.
                wvAB = cp.tile([C, C], bf16, tag="wvAB")
                nc.vector.tensor_scalar(out=wvAB[:], in0=wS[:, 2 * C:3 * C],
                                        scalar1=aT[:], scalar2=None, op0=OP.mult)
                vc_p = pre.tile([C, 2], f32, tag="gstats", name="vc_p")
                nc.tensor.matmul(vc_p[:, 0:1], wS[:, 2 * C:3 * C], bT[:],
                                 start=True, stop=True)
                vcS = sp_.tile([C, 1], f32, tag="vcS")
                nc.vector.tensor_copy(vcS[:], vc_p[:, 0:1])
                wov_p = pre.tile([C, 2], f32, tag="gstats", name="wov_p")
                nc.tensor.matmul(wov_p[:, 0:1], wS[:, 3 * C:4 * C], vcS[:],
                                 start=True, stop=True)
                beffT = sp_.tile([C, 1], f32, tag="beffT")
                nc.vector.tensor_tensor(out=beffT[:], in0=bS[:, 2:3],
                                        in1=wov_p[:, 0:1], op=OP.add)
                # h = a*x + b (f32r); first NPC chunks feed qt, the rest only
                # feed QK weights for late tiles so they can trail the vT copies
                for c in range(NPC):
                    nc.vector.tensor_scalar(out=hR[:, c * BNC:(c + 1) * BNC],
                                            in0=xS[:, c * BNC:(c + 1) * BNC],
                                            scalar1=aT[:], scalar2=bT[:],
                                            op0=OP.mult, op1=OP.add)

                # ---- projections ----
                for c in range(NPC):
                    # qt = (w_q^T w_k)^T h + w_k^T b_q;  scores = h^T qt
                    qtp = pre.tile([C, PCW], f32, tag="qtp")
                    nc.tensor.matmul(qtp[:], wR[:, 0:C], hR[:, PCW * c:PCW * (c + 1)],
                                     start=True, stop=True)
                    nc.scalar.activation(out=qtR[:, PCW * c:PCW * (c + 1)], in_=qtp[:],
                                         func=AF.Identity, bias=bS[:, 0:1], scale=1.0)
                VTW = min(1024, N)
                for g in range(N // VTW):
                    vtp = pvt.tile([C, VTW], f32, tag="vtp")
                    for jj in range(VTW // 128):
                        tj = (VTW // 128) * g + jj
                        nc.tensor.matmul(vtp[:, 128 * jj:128 * (jj + 1)],
                                         xB[:, 128 * tj:128 * (tj + 1)], wvAB[:],
                                         start=True, stop=True)
                    nc.vector.tensor_copy(vTR[:, VTW * g:VTW * (g + 1)], vtp[:])
                for c in range(NPC, N // BNC):
                    nc.vector.tensor_scalar(out=hR[:, c * BNC:(c + 1) * BNC],
                                            in0=xS[:, c * BNC:(c + 1) * BNC],
                                            scalar1=aT[:], scalar2=bT[:],
                                            op0=OP.mult, op1=OP.add)


            # ---- attention ----
            # sT double-buffered at STW wide (one exp op per tile). PE runs in
            # program order, so PV/ones for tile t are emitted one tile late:
            # while exp(t) runs on ACT, PE issues QK(t+1) instead of stalling.
            STW = min(1024, ICW)   # sT tile / exp chunk width
            NST = ICW // STW
            MMW = min(512, STW)    # matmul free-dim chunk
            NMM = STW // MMW
            NPAIR = NJT // 2  # rowsum matmuls run on pairwise P-sums (DVE adds)
            with tc.tile_pool(name="ps_sT", bufs=2, space="PSUM") as psT, \
                 tc.tile_pool(name="ps_rs", bufs=1, space="PSUM") as prs, \
                 tc.tile_pool(name="ps_h2", bufs=1, space="PSUM") as ph2:
                acc = {}        # ic -> (h2p, rsp)
                pend_pv = None  # (ic, odd t, Ppair) awaiting PV emission
                pend_ones = None  # (ic, pair_idx, Ps2) awaiting ones-MM emission

                def emit_pv(p):
                    # fp8 DoubleRow: one matmul contracts the pair of j-tiles
                    # (tp-1, tp); called only at odd tp.
                    icp, tp, Ppair = p
                    h2p = acc[icp][0]
                    pi = tp // 2
                    vpair = vTR[:, 256 * pi:256 * (pi + 1)].rearrange(
                        "p (two c) -> p two c", two=2)
                    for m in range(NMM):
                        nc.tensor.matmul(
                            h2p[:, m * MMW:(m + 1) * MMW], vpair,
                            Ppair[:, :, m * MMW:(m + 1) * MMW],
                            start=(pi == 0), stop=(pi == NJT // 2 - 1),
                            perf_mode=mybir.MatmulPerfMode.DoubleRow)

                def emit_ones(p, first=None, last=None, fp8=False):
                    icp, pi, Ps2p = p
                    rsp = acc[icp][1]
                    st = first if first is not None else (pi == 0)
                    sp2 = last if last is not None else False
                    lhs = onesF8[:] if fp8 else onesR[:]
                    for m in range(NMM):
                        nc.tensor.matmul(
                            rsp[:, m * MMW:(m + 1) * MMW], lhs,
                            Ps2p[:, m * MMW:(m + 1) * MMW],
                            start=st, stop=sp2)

                def finish_pass(ic):
                    h2p, rsp = acc[ic]
                    FCW = min(512, ICW)
                    for fc in range(ICW // FCW):
                        sl_i = slice(ic * ICW + fc * FCW, ic * ICW + (fc + 1) * FCW)
                        sl_f = slice(fc * FCW, (fc + 1) * FCW)
                        recipB = sp_.tile([C, FCW], f32, tag="recipB")
                        nc.vector.reciprocal_approx_fast(out=recipB[:], in_=rsp[:, sl_f])
                        nc.vector.tensor_tensor(out=h2nR[:, sl_i], in0=h2p[:, sl_f],
                                                in1=recipB[:], op=OP.mult)

                for ic in range(NIC):
                    acc[ic] = (ph2.tile([C, ICW], f32, tag="h2u", name=f"h2u{ic}"),
                               prs.tile([C, ICW], f32, tag="rs", name=f"rs{ic}"))
                    Ppair = None
                    for t in range(NJT):
                        for c2 in range(NST):
                            i0 = ic * ICW + c2 * STW
                            sT = psT.tile([C, STW], f32, tag="sT")
                            for m in range(NMM):
                                nc.tensor.matmul(
                                    sT[:, m * MMW:(m + 1) * MMW],
                                    hR[:, 128 * t:128 * (t + 1)],
                                    qtR[:, i0 + m * MMW:i0 + (m + 1) * MMW],
                                    start=True, stop=True)
                            if t % 2 == 0:
                                Ppair = pP.tile([C, 2, STW], f8, tag="P",
                                                name=f"P{ic}_{t}")
                            nc.scalar.activation(out=Ppair[:, t % 2, :], in_=sT[:],
                                                 func=AF.Exp, scale=SCALE)
                            if t % 2 == 1:
                                pend_pv = (ic, t, Ppair)
                                continue
                            if pend_pv is not None:
                                emit_pv(pend_pv)
                                tp = pend_pv[1]
                                Pp = pend_pv[2]
                                if tp == NJT - 1:
                                    # tail of the pass: direct fp8 ones-MMs so
                                    # the rowsum doesn't wait on a DVE pair-add
                                    if pend_ones is not None:
                                        emit_ones(pend_ones)
                                        pend_ones = None
                                    emit_ones((pend_pv[0], -1, Pp[:, 0, :]),
                                              first=False, last=False, fp8=True)
                                    emit_ones((pend_pv[0], -1, Pp[:, 1, :]),
                                              first=False, last=True, fp8=True)
                                    finish_pass(pend_pv[0])
                                else:
                                    # DVE pair-sum of the two P slices just used
                                    Ps2 = pP.tile([C, STW], f32r, tag="Ps2")
                                    nc.vector.tensor_tensor(
                                        out=Ps2[:], in0=Pp[:, 0, :],
                                        in1=Pp[:, 1, :], op=OP.add)
                                    if pend_ones is not None:
                                        emit_ones(pend_ones)
                                    pend_ones = (pend_pv[0], tp // 2, Ps2)
                                pend_pv = None
                emit_pv(pend_pv)
                tp = pend_pv[1]
                Pp = pend_pv[2]
                if pend_ones is not None:
                    emit_ones(pend_ones)
                    pend_ones = None
                emit_ones((pend_pv[0], -1, Pp[:, 0, :]), first=False, last=False,
                          fp8=True)
                emit_ones((pend_pv[0], -1, Pp[:, 1, :]), first=False, last=True,
                          fp8=True)
                finish_pass(pend_pv[0])

            # ---- out projection + bias + residual ----
            with tc.tile_pool(name="ps_ep", bufs=2, space="PSUM") as pep:
                for c in range(NPC):
                    pop = pep.tile([C, PCW], f32, tag="pop")
                    nc.tensor.matmul(pop[:], wR[:, 3 * C:4 * C],
                                     h2nR[:, PCW * c:PCW * (c + 1)], start=True, stop=True)
                    nc.vector.scalar_tensor_tensor(
                        out=outS[:, PCW * c:PCW * (c + 1)], in0=pop[:], scalar=beffT[:],
                        in1=xS[:, PCW * c:PCW * (c + 1)], op0=OP.add, op1=OP.add)
                    nc.sync.dma_start(o_d[:, PCW * c:PCW * (c + 1)],
                                      outS[:, PCW * c:PCW * (c + 1)])
            if _loop is not None:
                _loop.__exit__(None, None, None)

    nc.compile()
    return nc


def host_inputs(x, gn_w, gn_b, w_qkv, b_qkv, w_out, b_out):
    """Build the 8 per-core input maps from the full problem inputs."""
    x = np.asarray(x, dtype=np.float32)
    B, _, N = x.shape
    S = N // 2
    w_qkv = np.asarray(w_qkv, np.float32)
    w_out = np.asarray(w_out, np.float32)
    b_qkv = np.asarray(b_qkv, np.float32)
    b_out = np.asarray(b_out, np.float32)
    gn_w = np.asarray(gn_w, np.float32)
    gn_b = np.asarray(gn_b, np.float32)

    # scores = h^T (w_q^T w_k) h + h^T (w_k^T b_q); the k bias is
    # softmax-invariant and dropped, q/k are never materialized on device.
    M = w_qkv[0:C].T @ w_qkv[C:2 * C]
    wcat = np.concatenate(
        [M, np.zeros((C, C), np.float32), w_qkv[2 * C:3 * C].T, w_out.T],
        axis=1).astype(np.float32)   # [C, 4C]: [M, unused, w_v^T, w_out^T]
    gidx = np.arange(C) // GS
    gmask = (gidx[:, None] == gidx[None, :]).astype(np.float32) / GS
    b_eff = b_out + w_out @ b_qkv[2 * C:3 * C]
    bqt = w_qkv[C:2 * C].T @ b_qkv[0:C]
    bcat = np.stack([bqt, b_qkv[C:2 * C], b_eff, gn_w, gn_b], axis=1)
    bcat = np.ascontiguousarray(bcat, np.float32)       # [C, 5]

    in_maps = []
    for core in range(N_CORES):
        b, half = divmod(core, 2)
        xb = np.roll(x[b], -half * S, axis=1)
        in_maps.append({"x": np.ascontiguousarray(xb), "wcat": wcat,
                        "gmask": gmask, "bcat": bcat})
    return in_maps


_NC_CACHE = {}
_RUNNER_CACHE = {}


def _make_runner(nc):
    """Compile-once runner: replicates bass2jax.run_bass_via_pjrt but keeps the
    jitted sharded callable so repeat executions skip recompilation."""
    import jax
    import concourse.mybir as mybir
    from jax.sharding import Mesh, PartitionSpec
    from jax.experimental.shard_map import shard_map
    from concourse.bass2jax import (_bass_exec_p, install_neuronx_cc_hook,
                                    partition_id_tensor)

    install_neuronx_cc_hook()
    partition_name = nc.partition_id_tensor.name if nc.partition_id_tensor else None
    in_names, out_names, out_avals, zero_shapes = [], [], [], []
    for alloc in nc.m.functions[0].allocations:
        if not isinstance(alloc, mybir.MemoryLocationSet):
            continue
        name = alloc.memorylocations[0].name
        if alloc.kind == "ExternalInput":
            if name == partition_name:
                continue
            in_names.append(name)
        elif alloc.kind == "ExternalOutput":
            out_names.append(name)
            shape = tuple(alloc.tensor_shape)
            dtype = mybir.dt.np(alloc.dtype)
            out_avals.append(jax.core.ShapedArray(shape, dtype))
            zero_shapes.append((shape, dtype))
    n_params = len(in_names)
    all_names = in_names + out_names
    if partition_name is not None:
        all_names = all_names + [partition_name]
    donate = tuple(range(n_params, n_params + len(out_names)))

    def _body(*args):
        operands = list(args)
        if partition_name is not None:
            operands.append(partition_id_tensor())
        return tuple(_bass_exec_p.bind(
            *operands, out_avals=tuple(out_avals), in_names=tuple(all_names),
            out_names=tuple(out_names), lowering_input_output_aliases=(),
            sim_require_finite=True, sim_require_nnan=True, nc=nc))

    devices = jax.devices()[:N_CORES]
    mesh = Mesh(np.asarray(devices), ("core",))
    specs = (PartitionSpec("core"),)
    sharded = jax.jit(
        shard_map(_body, mesh=mesh,
                  in_specs=specs * (n_params + len(out_names)),
                  out_specs=specs * len(out_names), check_rep=False),
        donate_argnums=donate, keep_unused=True)

    def run(in_maps):
        concat_in = [np.concatenate([np.asarray(m[nm]) for m in in_maps], axis=0)
                     for nm in in_names]
        concat_zeros = [np.zeros((N_CORES * s[0], *s[1:]), d) for s, d in zero_shapes]
        out_arrs = sharded(*concat_in, *concat_zeros)
        out_arrs = [np.asarray(a) for a in out_arrs]
        return [{nm: out_arrs[i].reshape(N_CORES, *out_avals[i].shape)[c]
                 for i, nm in enumerate(out_names)} for c in range(N_CORES)]

    return run


def get_runner(N=4096):
    if N not in _RUNNER_CACHE:
        if N not in _NC_CACHE:
            _NC_CACHE[N] = build(N)
        _RUNNER_CACHE[N] = _make_runner(_NC_CACHE[N])
    return _RUNNER_CACHE[N]


def kernel(x, gn_w, gn_b, w_qkv, b_qkv, w_out, b_out):
    from concourse._compat import axon_active

    x = np.asarray(x, dtype=np.float32)
    B, _, N = x.shape
    S = N // 2
    in_maps = host_inputs(x, gn_w, gn_b, w_qkv, b_qkv, w_out, b_out)
    if axon_active():
        results = get_runner(N)(in_maps)
    else:
        from concourse.bass_utils import run_bass_kernel_spmd

        if N not in _NC_CACHE:
            _NC_CACHE[N] = build(N)
        results = run_bass_kernel_spmd(_NC_CACHE[N], in_maps,
                                       core_ids=list(range(N_CORES))).results
    out = np.empty((B, C, N), dtype=np.float32)
    for core in range(N_CORES):
        b, half = divmod(core, 2)
        out[b, :, half * S:(half + 1) * S] = results[core]["out"]
    return out
